# revision 22
# baseline (speedup 1.0000x reference)
"""Trainium2 Bass kernel for a 2-layer LSTM (B=1024, T=512, D=32, H=64) + MLP head.

Strategy (per core, data-parallel over batch: B_local = 128):
  * All state kept "transposed": [feature-rows on partitions, batch on free dim].
  * Wavefront over t: one merged step k processes layer0 at t=k and layer1 at
    t=k-1.  Layer0 state lives on partitions 0:64, layer1 on 64:128, so every
    elementwise op covers both layers in a single [128, 128] instruction.
  * Recurrent state is ONE tile HH [128, BL]: rows 0:64 = 2*h0, rows 64:128 =
    2*h1.  Per gate g the pre-activation z[:, g*128:(g+1)*128] (partitions
    0:64 = layer0 gate, 64:128 = layer1 gate) is computed by TWO matmuls:
      - MM_h: lhsT = WH[:, 128g:128g+128]  (K=128: both layers' h-inputs,
        M=128: both layers' gate outputs) against HH  -> full 128x128 PE array
      - MM_x: lhsT = WX[0:33, ...] (x_t rows 0:32 + bias row 32) against X
    8 matmuls (bf16) per step total.
  * One ACT op computes s = tanh(0.5*z) per stream.  Sigmoid gates use
    sigma(z) = (tanh(z/2)+1)/2; the g gate's weights/bias are pre-doubled on
    the host so tanh(0.5 * 2g) = tanh(g) exactly.
  * Cell update in 3 scalar_tensor_tensor ops using scaled state C^ = 2c:
      P = (s_f + 1) * C^         (= 2*sigma(f)*C^)
      Q = (s_i + 1) * s_g        (= 2*sigma(i)*tanh(g))
      C^' = 0.5*P + Q            (= 2*c')
    th = tanh(0.5*C^') = tanh(c'), and HH' = (s_o + 1)*th = 2h in a single
    op covering both layers.  All h-consuming weights are pre-halved on the
    host (exact in fp32).
  * TWO phase-shifted half-batch streams (A: batch 0:64, B: 64:128) run the
    serial per-step chain MM -> ACT(s) -> DVE(P,Q,C) -> ACT(th) -> DVE(h)
    interleaved, so each engine works on one stream while the other stream's
    chain is on a different engine.  The x/bias matmuls cover the full batch
    once per step and run ahead of the recurrence (4-deep PSUM rotation).
  * Matmul operands are bf16 (fp32 matmul runs at 1/4 PE rate); PSUM
    accumulation and the gate/cell elementwise chain stay fp32.
  * PSUM gotcha encoded here: matmul start=True resets has_written bits for
    the WHOLE bank, so exactly one matmul per step carries start=True.
"""

import numpy as np
import ml_dtypes
from contextlib import ExitStack

import concourse.bass as bass
import concourse.bacc as bacc
import concourse.mybir as mybir
import concourse.tile as tile
from concourse.bass_utils import run_bass_kernel_spmd

F32 = mybir.dt.float32
BF16 = mybir.dt.bfloat16
NP_BF16 = ml_dtypes.bfloat16
AT = mybir.ActivationFunctionType
OP = mybir.AluOpType

B, T, D, H = 1024, 512, 32, 64
N_CORES = 8
BL = B // N_CORES  # 128 batch per core


def build_nc(t_steps=T):
    nc = bacc.Bacc()

    xT = nc.declare_dram_parameter("xT", [t_steps, D, BL], BF16, isOutput=False)
    whd = nc.declare_dram_parameter("wh", [128, 512], BF16, isOutput=False)
    wxd = nc.declare_dram_parameter("wx", [33, 512], BF16, isOutput=False)
    hw1d = nc.declare_dram_parameter("hw1", [65, 32], BF16, isOutput=False)
    hw2d = nc.declare_dram_parameter("hw2", [33, 1], BF16, isOutput=False)
    yd = nc.declare_dram_parameter("y", [1, BL], F32, isOutput=True)

    HB = BL // 2  # 64: batch per stream

    with tile.TileContext(nc) as tc, ExitStack() as ctx:
        const = ctx.enter_context(tc.tile_pool(name="const", bufs=1))
        st = ctx.enter_context(tc.tile_pool(name="state", bufs=1))
        ps = ctx.enter_context(tc.tile_pool(name="ps", bufs=4, space="PSUM"))

        # ---- weights into SBUF ----
        # DMA into staging, then DVE-copy into the real tiles.  This funnels
        # every init dependency through the single DVE processor, keeping all
        # downstream instructions within the HW per-instruction sync-wait
        # budget (the DMA queues are distinct "processors" otherwise).
        whs = const.tile([128, 512], BF16)
        nc.sync.dma_start(whs[:, :], whd[:, :])
        wxs = const.tile([64, 512], BF16)
        nc.sync.dma_start(wxs[0:33, :], wxd[:, :])
        hw1s = const.tile([128, 32], BF16)
        nc.sync.dma_start(hw1s[0:65, :], hw1d[:, :])
        hw2s = const.tile([128, 1], BF16)
        nc.sync.dma_start(hw2s[0:33, :], hw2d[:, :])
        wh = const.tile([128, 512], BF16)
        nc.vector.tensor_copy(wh[:, :], whs[:, :])
        wx = const.tile([64, 512], BF16)
        nc.vector.tensor_copy(wx[0:33, :], wxs[0:33, :])
        hw1 = const.tile([128, 32], BF16)
        nc.vector.tensor_copy(hw1[0:65, :], hw1s[0:65, :])
        hw2 = const.tile([128, 1], BF16)
        nc.vector.tensor_copy(hw2[0:33, :], hw2s[0:33, :])

        # ---- persistent state (manually double-buffered), per stream ----
        # Two phase-shifted half-batch streams (A: batch 0:64, B: 64:128)
        # pipeline the serial per-step chain across the engines.
        # HH: rows 0:64 = 2*h0, rows 64:128 = 2*h1 (bf16, matmul operand)
        HH = [[st.tile([128, HB], BF16, name=f"HH_{s}_{i}") for i in range(2)]
              for s in range(2)]
        # X: rows 0:32 = x_t, row 32 = ones (bias row); shared by streams
        X = [st.tile([64, BL], BF16, name=f"X_{i}") for i in range(2)]
        # C: scaled cell state 2*c, layer0 rows 0:64, layer1 rows 64:128
        S = [[st.tile([128, 192], F32, name=f"S_{s}_{i}") for i in range(2)]
             for s in range(2)]
        # TH: cols 0:64 = tanh(0.5*C'), cols 64:128 = s_o (one strided ACT)
        TH = [[st.tile([128, 128], F32, name=f"TH_{s}_{i}") for i in range(2)]
              for s in range(2)]
        Z0 = st.tile([128, HB], F32, name="Z0")
        PP = [st.tile([128, HB], F32, name=f"PP_{s}") for s in range(2)]
        QQ = [st.tile([128, HB], F32, name=f"QQ_{s}") for s in range(2)]

        nc.vector.memset(Z0[:, :], 0.0)
        for i in range(2):
            nc.vector.memset(X[i][32:33, :], 1.0)
            for s in range(2):
                nc.vector.memset(HH[s][i][:, :], 0.0)

        nc.sync.dma_start(X[0][0:32, :], xT[0])

        def mm_h(z, sig, cur, g):
            # h-recurrence piece for stream sig, gate g: full 128x128 lhsT
            nc.tensor.matmul(
                z[0:128, g * 128 + 64 * sig:g * 128 + 64 * sig + 64],
                wh[0:128, g * 128:(g + 1) * 128],
                HH[sig][cur][0:128, :],
                start=False, stop=True,
            )

        def chain_a(z, z3, prev_z, sig, cur, nxt, k):
            # gate activations for i,f,g only: the o gate's tanh rides the
            # th-ACT (below), so s waits on 3 matmuls, not 4
            s3 = S[sig][cur][0:128, 0:192].rearrange("p (g b) -> p g b", g=3)
            nc.scalar.activation(s3, z3[:, 0:3, 64 * sig:64 * sig + 64],
                                 AT.Tanh, bias=0.0, scale=0.5)
            s = S[sig][cur]
            # C' lives in the z-bank's (dead after s) g-gate columns: the
            # previous step's bank holds C'(k-1) for P, this step's receives
            # C'(k) so the th-ACT can read [C' | z_o] in ONE strided op.
            co = 256 + 64 * sig
            cprev = Z0[:, :] if k == 0 else prev_z[0:128, co:co + 64]
            # P = (s_f + 1) * C_prev ; Q = (s_i + 1) * s_g ; C' = 0.5P + Q
            nc.vector.scalar_tensor_tensor(
                PP[sig][:, :], s[:, 64:128], 1.0, cprev,
                op0=OP.add, op1=OP.mult,
            )
            nc.vector.scalar_tensor_tensor(
                QQ[sig][:, :], s[:, 0:64], 1.0, s[:, 128:192],
                op0=OP.add, op1=OP.mult,
            )
            # k=0: restrict to layer0 rows so layer1's cell state stays
            # exactly 0 for its first real step at k=1
            r1 = 64 if k == 0 else 128
            nc.vector.scalar_tensor_tensor(
                z[0:r1, co:co + 64], PP[sig][0:r1, :], 0.5, QQ[sig][0:r1, :],
                op0=OP.mult, op1=OP.add,
            )
            if k == 0:
                nc.vector.memset(z[64:128, co:co + 64], 0.0)

        def chain_b(z3, sig, cur, nxt):
            th = TH[sig][cur]
            # one ACT: cols 0:64 = tanh(0.5 C') , cols 64:128 = s_o
            t3 = th[0:128, 0:128].rearrange("p (g b) -> p g b", g=2)
            nc.scalar.activation(t3, z3[:, 2:4, 64 * sig:64 * sig + 64],
                                 AT.Tanh, bias=0.0, scale=0.5)
            # 2*h for both layers -> state tile for step k+1
            nc.vector.scalar_tensor_tensor(
                HH[sig][nxt][:, :], th[:, 64:128], 1.0, th[:, 0:64],
                op0=OP.add, op1=OP.mult,
            )

        # ---- recurrence ----
        # Step k: layer0 at t=k, layer1 at t=k-1 (wavefront; see 2a notes).
        # Emission order interleaves the streams so the ACT FIFO runs
        # s_A, s_B, th_A, th_B and the DVE FIFO runs
        # P_A,Q_A,C_A, hm_A, P_B,Q_B,C_B, hm_B per step.
        prev_z = None
        for k in range(t_steps + 1):
            cur, nxt = k % 2, (k + 1) % 2

            z = ps.tile([128, 512], F32, name="z", tag="z")
            z3 = z[0:128, 0:512].rearrange("p (g b) -> p g b", g=4)
            # x + bias part: full batch, both streams, off the critical path.
            # start=True resets has_written for the WHOLE bank, so only the
            # first matmul of the step may carry it; later blocks see
            # has_written=0 and overwrite (fresh write) as needed.
            for g in range(4):
                nc.tensor.matmul(
                    z[0:128, g * 128:(g + 1) * 128],
                    wx[0:33, g * 128:(g + 1) * 128],
                    X[cur][0:33, :],
                    start=(g == 0), stop=False,
                )
            for g in range(4):
                mm_h(z, 0, cur, g)
            chain_a(z, z3, prev_z, 0, cur, nxt, k)
            for g in range(4):
                mm_h(z, 1, cur, g)
            chain_a(z, z3, prev_z, 1, cur, nxt, k)
            chain_b(z3, 0, cur, nxt)
            chain_b(z3, 1, cur, nxt)
            prev_z = z
            if k + 1 < t_steps:
                nc.sync.dma_start(X[nxt][0:32, :], xT[k + 1])

        # ---- head: y = W2 @ relu(W1 @ h1 + b1) + b2 ----
        hd = st.tile([128, BL], BF16)
        nc.vector.memset(hd[64:65, :], 1.0)
        # move 2*h1 from partitions 64:128 down to 0:64 (partition shift: DMA)
        fin = (t_steps + 1) % 2
        nc.sync.dma_start(hd[0:64, 0:HB], HH[0][fin][64:128, :])
        nc.sync.dma_start(hd[0:64, HB:BL], HH[1][fin][64:128, :])
        ph = ps.tile([128, BL], F32, name="ph", tag="ph", bufs=1)
        nc.tensor.matmul(ph[0:32, :], hw1[0:65, 0:32], hd[0:65, :],
                         start=True, stop=True)
        hr = st.tile([128, BL], BF16)
        nc.vector.memset(hr[32:33, :], 1.0)
        nc.scalar.activation(hr[0:32, :], ph[0:32, :], AT.Relu)
        po = ps.tile([128, BL], F32, name="po", tag="po", bufs=1)
        nc.tensor.matmul(po[0:1, :], hw2[0:33, 0:1], hr[0:33, :],
                         start=True, stop=True)
        ysb = st.tile([1, BL], F32)
        nc.scalar.copy(ysb[0:1, :], po[0:1, :])
        nc.sync.dma_start(yd[:, :], ysb[0:1, :])

    return nc


def prep_weights(Wih0, Whh0, bih0, bhh0, Wih1, Whh1, bih1, bhh1, W1, b1, W2, b2):
    """Host-side weight re-layout.  Gate order i,f,g,o (torch LSTM order).

    Scalings (all exact powers of two in fp32):
      * h-input columns are halved (state is stored as 2*h),
      * the g gate's whole block (weights + bias) is doubled so that the
        uniform tanh(0.5*z) activation yields exactly tanh(g).
    """
    f32 = np.float32
    bias0 = (bih0 + bhh0).astype(f32)
    bias1 = (bih1 + bhh1).astype(f32)
    wh = np.zeros((128, 512), f32)
    wx = np.zeros((33, 512), f32)
    for g in range(4):
        rs = slice(g * 64, (g + 1) * 64)
        c0 = slice(g * 128, g * 128 + 64)        # layer0 gate-g out columns
        c1 = slice(g * 128 + 64, (g + 1) * 128)  # layer1 gate-g out columns
        sc = 2.0 if g == 2 else 1.0
        wh[0:64, c0] = Whh0[rs, :].T * (0.5 * sc)
        wh[0:64, c1] = Wih1[rs, :].T * (0.5 * sc)
        wh[64:128, c1] = Whh1[rs, :].T * (0.5 * sc)
        wx[0:32, c0] = Wih0[rs, :].T * sc
        wx[32, c0] = bias0[rs] * sc
        wx[32, c1] = bias1[rs] * sc
    hw1 = np.zeros((65, 32), f32)
    hw1[0:64, :] = W1.T * 0.5
    hw1[64, :] = b1
    hw2 = np.zeros((33, 1), f32)
    hw2[0:32, :] = W2.T
    hw2[32, :] = b2
    return (wh.astype(NP_BF16), wx.astype(NP_BF16),
            hw1.astype(NP_BF16), hw2.astype(NP_BF16))


_NC_CACHE = {}


def _get_nc(t_steps):
    if t_steps not in _NC_CACHE:
        nc = build_nc(t_steps)
        if not nc.is_finalized():
            nc.finalize()
        _NC_CACHE[t_steps] = nc
    return _NC_CACHE[t_steps]


def run(x, weights, t_steps=T, trace=False):
    """x: [B, t_steps, D] float32; weights: tuple from prep_weights."""
    wh, wx, hw1, hw2 = weights
    nc = _get_nc(t_steps)
    xs = np.ascontiguousarray(x.transpose(1, 2, 0).astype(NP_BF16))  # [T, D, B]
    in_maps = []
    for c in range(N_CORES):
        in_maps.append({
            "xT": np.ascontiguousarray(xs[:, :, c * BL:(c + 1) * BL]),
            "wh": wh, "wx": wx, "hw1": hw1, "hw2": hw2,
        })
    res = run_bass_kernel_spmd(nc, in_maps, core_ids=list(range(N_CORES)),
                               trace=trace)
    y = np.concatenate([res.results[c]["y"][0] for c in range(N_CORES)])
    return y, res


def kernel(x, Wih0, Whh0, bih0, bhh0, Wih1, Whh1, bih1, bhh1, W1, b1, W2, b2):
    weights = prep_weights(
        np.asarray(Wih0, np.float32), np.asarray(Whh0, np.float32),
        np.asarray(bih0, np.float32), np.asarray(bhh0, np.float32),
        np.asarray(Wih1, np.float32), np.asarray(Whh1, np.float32),
        np.asarray(bih1, np.float32), np.asarray(bhh1, np.float32),
        np.asarray(W1, np.float32), np.asarray(b1, np.float32),
        np.asarray(W2, np.float32), np.asarray(b2, np.float32),
    )
    y, _ = run(np.asarray(x, np.float32), weights)
    return y


# revision 23
# speedup vs baseline: 1.6194x; 1.6194x over previous
"""Trainium2 Bass kernel for a 2-layer LSTM (B=1024, T=512, D=32, H=64) + MLP head.

Strategy (per core, data-parallel over batch: B_local = 128):
  * All state kept "transposed": [feature-rows on partitions, batch on free dim].
  * Wavefront over t: one merged step k processes layer0 at t=k and layer1 at
    t=k-1.  Layer0 state lives on partitions 0:64, layer1 on 64:128, so every
    elementwise op covers both layers in a single [128, 128] instruction.
  * Recurrent state is ONE tile HH [128, BL]: rows 0:64 = 2*h0, rows 64:128 =
    2*h1.  Per gate g the pre-activation z[:, g*128:(g+1)*128] (partitions
    0:64 = layer0 gate, 64:128 = layer1 gate) is computed by TWO matmuls:
      - MM_h: lhsT = WH[:, 128g:128g+128]  (K=128: both layers' h-inputs,
        M=128: both layers' gate outputs) against HH  -> full 128x128 PE array
      - MM_x: lhsT = WX[0:33, ...] (x_t rows 0:32 + bias row 32) against X
    8 matmuls (bf16) per step total.
  * One ACT op computes s = tanh(0.5*z) per stream.  Sigmoid gates use
    sigma(z) = (tanh(z/2)+1)/2; the g gate's weights/bias are pre-doubled on
    the host so tanh(0.5 * 2g) = tanh(g) exactly.
  * Cell update in 3 scalar_tensor_tensor ops using scaled state C^ = 2c:
      P = (s_f + 1) * C^         (= 2*sigma(f)*C^)
      Q = (s_i + 1) * s_g        (= 2*sigma(i)*tanh(g))
      C^' = 0.5*P + Q            (= 2*c')
    th = tanh(0.5*C^') = tanh(c'), and HH' = (s_o + 1)*th = 2h in a single
    op covering both layers.  All h-consuming weights are pre-halved on the
    host (exact in fp32).
  * TWO phase-shifted half-batch streams (A: batch 0:64, B: 64:128) run the
    serial per-step chain MM -> ACT(s) -> DVE(P,Q,C) -> ACT(th) -> DVE(h)
    interleaved, so each engine works on one stream while the other stream's
    chain is on a different engine.  The x/bias matmuls cover the full batch
    once per step and run ahead of the recurrence (4-deep PSUM rotation).
  * Matmul operands are bf16 (fp32 matmul runs at 1/4 PE rate); PSUM
    accumulation and the gate/cell elementwise chain stay fp32.
  * PSUM gotcha encoded here: matmul start=True resets has_written bits for
    the WHOLE bank, so exactly one matmul per step carries start=True.
"""

import numpy as np
import ml_dtypes
from contextlib import ExitStack

import concourse.bass as bass
import concourse.bacc as bacc
import concourse.mybir as mybir
import concourse.tile as tile
from concourse.bass_utils import run_bass_kernel_spmd

F32 = mybir.dt.float32
BF16 = mybir.dt.bfloat16
NP_BF16 = ml_dtypes.bfloat16
AT = mybir.ActivationFunctionType
OP = mybir.AluOpType

B, T, D, H = 1024, 512, 32, 64
N_CORES = 8
BL = B // N_CORES  # 128 batch per core


def build_nc(t_steps=T):
    nc = bacc.Bacc()

    xT = nc.declare_dram_parameter("xT", [t_steps, D, BL], BF16, isOutput=False)
    whd = nc.declare_dram_parameter("wh", [128, 512], BF16, isOutput=False)
    wxd = nc.declare_dram_parameter("wx", [33, 512], BF16, isOutput=False)
    hw1d = nc.declare_dram_parameter("hw1", [65, 32], BF16, isOutput=False)
    hw2d = nc.declare_dram_parameter("hw2", [33, 1], BF16, isOutput=False)
    yd = nc.declare_dram_parameter("y", [1, BL], F32, isOutput=True)

    HB = BL // 2  # 64: batch per stream

    with tile.TileContext(nc) as tc, ExitStack() as ctx:
        const = ctx.enter_context(tc.tile_pool(name="const", bufs=1))
        st = ctx.enter_context(tc.tile_pool(name="state", bufs=1))
        ps = ctx.enter_context(tc.tile_pool(name="ps", bufs=4, space="PSUM"))

        # ---- weights into SBUF ----
        # DMA into staging, then DVE-copy into the real tiles.  This funnels
        # every init dependency through the single DVE processor, keeping all
        # downstream instructions within the HW per-instruction sync-wait
        # budget (the DMA queues are distinct "processors" otherwise).
        whs = const.tile([128, 512], BF16)
        nc.sync.dma_start(whs[:, :], whd[:, :])
        wxs = const.tile([64, 512], BF16)
        nc.sync.dma_start(wxs[0:33, :], wxd[:, :])
        hw1s = const.tile([128, 32], BF16)
        nc.sync.dma_start(hw1s[0:65, :], hw1d[:, :])
        hw2s = const.tile([128, 1], BF16)
        nc.sync.dma_start(hw2s[0:33, :], hw2d[:, :])
        wh = const.tile([128, 512], BF16)
        nc.vector.tensor_copy(wh[:, :], whs[:, :])
        wx = const.tile([64, 512], BF16)
        nc.vector.tensor_copy(wx[0:33, :], wxs[0:33, :])
        hw1 = const.tile([128, 32], BF16)
        nc.vector.tensor_copy(hw1[0:65, :], hw1s[0:65, :])
        hw2 = const.tile([128, 1], BF16)
        nc.vector.tensor_copy(hw2[0:33, :], hw2s[0:33, :])

        # ---- persistent state (manually double-buffered), per stream ----
        # Two phase-shifted half-batch streams (A: batch 0:64, B: 64:128)
        # pipeline the serial per-step chain across the engines.
        # HH: rows 0:64 = 2*h0, rows 64:128 = 2*h1 (bf16, matmul operand)
        HH = [[st.tile([128, HB], BF16, name=f"HH_{s}_{i}") for i in range(2)]
              for s in range(2)]
        # X: rows 0:32 = x_t, row 32 = ones (bias row); shared by streams
        X = [st.tile([64, BL], BF16, name=f"X_{i}") for i in range(2)]
        # C: scaled cell state 2*c, layer0 rows 0:64, layer1 rows 64:128
        C = [[st.tile([128, HB], F32, name=f"C_{s}_{i}") for i in range(2)]
             for s in range(2)]
        S = [[st.tile([128, 256], F32, name=f"S_{s}_{i}") for i in range(2)]
             for s in range(2)]
        TH = [[st.tile([128, HB], F32, name=f"TH_{s}_{i}") for i in range(2)]
              for s in range(2)]
        PP = [st.tile([128, HB], F32, name=f"PP_{s}") for s in range(2)]
        QQ = [st.tile([128, HB], F32, name=f"QQ_{s}") for s in range(2)]

        for i in range(2):
            nc.vector.memset(X[i][32:33, :], 1.0)
            for s in range(2):
                nc.vector.memset(HH[s][i][:, :], 0.0)
                nc.vector.memset(C[s][i][:, :], 0.0)

        nc.sync.dma_start(X[0][0:32, :], xT[0])

        def mm_h(z, sig, cur, g):
            # h-recurrence piece for stream sig, gate g: full 128x128 lhsT
            nc.tensor.matmul(
                z[0:128, g * 128 + 64 * sig:g * 128 + 64 * sig + 64],
                wh[0:128, g * 128:(g + 1) * 128],
                HH[sig][cur][0:128, :],
                start=False, stop=True,
            )

        def chain_a(z3, sig, cur, nxt, k):
            # gate activations: s = tanh(0.5 z) for this stream's columns
            s3 = S[sig][cur][0:128, 0:256].rearrange("p (g b) -> p g b", g=4)
            nc.scalar.activation(s3, z3[:, :, 64 * sig:64 * sig + 64],
                                 AT.Tanh, bias=0.0, scale=0.5)
            s = S[sig][cur]
            # P = (s_f + 1) * C_prev ; Q = (s_i + 1) * s_g ; C' = 0.5P + Q
            nc.vector.scalar_tensor_tensor(
                PP[sig][:, :], s[:, 64:128], 1.0, C[sig][nxt][:, :],
                op0=OP.add, op1=OP.mult,
            )
            nc.vector.scalar_tensor_tensor(
                QQ[sig][:, :], s[:, 0:64], 1.0, s[:, 128:192],
                op0=OP.add, op1=OP.mult,
            )
            # k=0: restrict to layer0 rows so layer1's cell state stays
            # exactly 0 for its first real step at k=1
            r1 = 64 if k == 0 else 128
            nc.vector.scalar_tensor_tensor(
                C[sig][cur][0:r1, :], PP[sig][0:r1, :], 0.5, QQ[sig][0:r1, :],
                op0=OP.mult, op1=OP.add,
            )

        def chain_b(sig, cur, nxt):
            s = S[sig][cur]
            th = TH[sig][cur]
            nc.scalar.activation(th[:, :], C[sig][cur][:, :], AT.Tanh,
                                 bias=0.0, scale=0.5)
            # 2*h for both layers -> state tile for step k+1
            nc.vector.scalar_tensor_tensor(
                HH[sig][nxt][:, :], s[:, 192:256], 1.0, th[:, :],
                op0=OP.add, op1=OP.mult,
            )

        # ---- recurrence ----
        # Step k: layer0 at t=k, layer1 at t=k-1 (wavefront; see 2a notes).
        # Emission order interleaves the streams so the ACT FIFO runs
        # s_A, s_B, th_A, th_B and the DVE FIFO runs
        # P_A,Q_A,C_A, hm_A, P_B,Q_B,C_B, hm_B per step.
        for k in range(t_steps + 1):
            cur, nxt = k % 2, (k + 1) % 2

            z = ps.tile([128, 512], F32, name="z", tag="z")
            z3 = z[0:128, 0:512].rearrange("p (g b) -> p g b", g=4)
            # x + bias part: full batch, both streams, off the critical path.
            # start=True resets has_written for the WHOLE bank, so only the
            # first matmul of the step may carry it; later blocks see
            # has_written=0 and overwrite (fresh write) as needed.
            for g in range(4):
                nc.tensor.matmul(
                    z[0:128, g * 128:(g + 1) * 128],
                    wx[0:33, g * 128:(g + 1) * 128],
                    X[cur][0:33, :],
                    start=(g == 0), stop=False,
                )
            for g in range(4):
                mm_h(z, 0, cur, g)
            chain_a(z3, 0, cur, nxt, k)
            for g in range(4):
                mm_h(z, 1, cur, g)
            chain_a(z3, 1, cur, nxt, k)
            chain_b(0, cur, nxt)
            chain_b(1, cur, nxt)
            if k + 1 < t_steps:
                nc.sync.dma_start(X[nxt][0:32, :], xT[k + 1])

        # ---- head: y = W2 @ relu(W1 @ h1 + b1) + b2 ----
        hd = st.tile([128, BL], BF16)
        nc.vector.memset(hd[64:65, :], 1.0)
        # move 2*h1 from partitions 64:128 down to 0:64 (partition shift: DMA)
        fin = (t_steps + 1) % 2
        nc.sync.dma_start(hd[0:64, 0:HB], HH[0][fin][64:128, :])
        nc.sync.dma_start(hd[0:64, HB:BL], HH[1][fin][64:128, :])
        ph = ps.tile([128, BL], F32, name="ph", tag="ph", bufs=1)
        nc.tensor.matmul(ph[0:32, :], hw1[0:65, 0:32], hd[0:65, :],
                         start=True, stop=True)
        hr = st.tile([128, BL], BF16)
        nc.vector.memset(hr[32:33, :], 1.0)
        nc.scalar.activation(hr[0:32, :], ph[0:32, :], AT.Relu)
        po = ps.tile([128, BL], F32, name="po", tag="po", bufs=1)
        nc.tensor.matmul(po[0:1, :], hw2[0:33, 0:1], hr[0:33, :],
                         start=True, stop=True)
        ysb = st.tile([1, BL], F32)
        nc.scalar.copy(ysb[0:1, :], po[0:1, :])
        nc.sync.dma_start(yd[:, :], ysb[0:1, :])

    return nc


def prep_weights(Wih0, Whh0, bih0, bhh0, Wih1, Whh1, bih1, bhh1, W1, b1, W2, b2):
    """Host-side weight re-layout.  Gate order i,f,g,o (torch LSTM order).

    Scalings (all exact powers of two in fp32):
      * h-input columns are halved (state is stored as 2*h),
      * the g gate's whole block (weights + bias) is doubled so that the
        uniform tanh(0.5*z) activation yields exactly tanh(g).
    """
    f32 = np.float32
    bias0 = (bih0 + bhh0).astype(f32)
    bias1 = (bih1 + bhh1).astype(f32)
    wh = np.zeros((128, 512), f32)
    wx = np.zeros((33, 512), f32)
    for g in range(4):
        rs = slice(g * 64, (g + 1) * 64)
        c0 = slice(g * 128, g * 128 + 64)        # layer0 gate-g out columns
        c1 = slice(g * 128 + 64, (g + 1) * 128)  # layer1 gate-g out columns
        sc = 2.0 if g == 2 else 1.0
        wh[0:64, c0] = Whh0[rs, :].T * (0.5 * sc)
        wh[0:64, c1] = Wih1[rs, :].T * (0.5 * sc)
        wh[64:128, c1] = Whh1[rs, :].T * (0.5 * sc)
        wx[0:32, c0] = Wih0[rs, :].T * sc
        wx[32, c0] = bias0[rs] * sc
        wx[32, c1] = bias1[rs] * sc
    hw1 = np.zeros((65, 32), f32)
    hw1[0:64, :] = W1.T * 0.5
    hw1[64, :] = b1
    hw2 = np.zeros((33, 1), f32)
    hw2[0:32, :] = W2.T
    hw2[32, :] = b2
    return (wh.astype(NP_BF16), wx.astype(NP_BF16),
            hw1.astype(NP_BF16), hw2.astype(NP_BF16))


_NC_CACHE = {}


def _get_nc(t_steps):
    if t_steps not in _NC_CACHE:
        nc = build_nc(t_steps)
        if not nc.is_finalized():
            nc.finalize()
        _NC_CACHE[t_steps] = nc
    return _NC_CACHE[t_steps]


def run(x, weights, t_steps=T, trace=False):
    """x: [B, t_steps, D] float32; weights: tuple from prep_weights."""
    wh, wx, hw1, hw2 = weights
    nc = _get_nc(t_steps)
    xs = np.ascontiguousarray(x.transpose(1, 2, 0).astype(NP_BF16))  # [T, D, B]
    in_maps = []
    for c in range(N_CORES):
        in_maps.append({
            "xT": np.ascontiguousarray(xs[:, :, c * BL:(c + 1) * BL]),
            "wh": wh, "wx": wx, "hw1": hw1, "hw2": hw2,
        })
    res = run_bass_kernel_spmd(nc, in_maps, core_ids=list(range(N_CORES)),
                               trace=trace)
    y = np.concatenate([res.results[c]["y"][0] for c in range(N_CORES)])
    return y, res


def kernel(x, Wih0, Whh0, bih0, bhh0, Wih1, Whh1, bih1, bhh1, W1, b1, W2, b2):
    weights = prep_weights(
        np.asarray(Wih0, np.float32), np.asarray(Whh0, np.float32),
        np.asarray(bih0, np.float32), np.asarray(bhh0, np.float32),
        np.asarray(Wih1, np.float32), np.asarray(Whh1, np.float32),
        np.asarray(bih1, np.float32), np.asarray(bhh1, np.float32),
        np.asarray(W1, np.float32), np.asarray(b1, np.float32),
        np.asarray(W2, np.float32), np.asarray(b2, np.float32),
    )
    y, _ = run(np.asarray(x, np.float32), weights)
    return y
